# revision 17
# baseline (speedup 1.0000x reference)
"""MetaFeatureExtractor Trainium2 kernel (bf16 hot path).

Computes per-sample statistics over the time axis of x [B, T, C]:
  out = concat([mean, std(ddof=1), max, min, slope], axis=1) -> [B, 5C]

Sharding: pure data parallel over 8 NeuronCores (B=256 -> 32 samples/core).

Per-core layout: 8 cast-groups of 4 samples; group g's sample s is loaded as
SBUF [128 partitions, 16 j, 64 c] where partition p holds T-rows
[16p, 16p+16) -> 4 KiB contiguous DMA runs.  All compute runs in bf16
(tolerance is 2e-2; bf16 keeps norm rel err ~1e-3):

  ACT : f32->bf16 cast for half the groups, Square (bf16) for all
        groups, PSUM extraction, sqrt tail        (~52 us @ 1.2 GHz)
  DVE : f32->bf16 cast for the other half (2x_2p copy mode), max/min
        pairwise trees in bf16 (2x_1p tensor_tensor, max/min chains
        interleaved so the dependent levels pipeline), min negation,
        output tail                               (~51 us @ 0.96 GHz)
  PE  : sum(x), sum(x^2) via bf16 ones-matmuls into PSUM [1,512]
        per 8-sample pair                         (~27 us @ 2.4 GHz)
  GPSIMD: one bf16 partition_all_reduce(max) per 8-sample pair over
        the combined [max | -min] partials        (~16 us)
  DMA : 16.8 MB/core HBM->SBUF on the SP HWDGE queue; the endpoint
        load rides the ACT queue and all small result scatters ride the
        GPSIMD SWDGE ring so no queue head blocks across loop iterations.

Max/min are exact in bf16-rounded values; sums are fp32-accumulated.
"""

import threading

import numpy as np

B_TOTAL = 256
N_CORES = 8
B = B_TOTAL // N_CORES  # 32 samples per core
T = 2048
C = 64
S_GRP = 4                   # samples per cast-group
N_GRP = B // S_GRP          # 8 cast-groups
S_TREE = 2 * S_GRP          # samples per tree-group (8)
N_TREE = B // S_TREE        # 4 tree-groups
J = 16                      # T-rows per partition
P = 128                     # partitions
OUT_COLS = 5 * C            # 320

_cache = threading.local()


def _build(
    do_endpoint=True,
    do_cast=True,
    do_reduce=True,
    do_mm=True,
    do_par=True,
    n_grps=N_GRP,
    rep=1,
    loop_n=0,
    dve_cast=(1, 3, 5, 7),
    dve_sq=(0,),
    split_last_tree=False,
    batched_load=True,
    stat_bufs=4,
    xt_bufs=4,
    dma_split=False,
    slope_top=True,
    mm_q7dma=True,
    srq_q7dma=True,
    y_q7dma=False,
    e_q7dma=False,
    x2_bufs=2,
    rows_bufs=2,
):
    import concourse.bacc as bacc
    import concourse.bass as bass
    import concourse.tile as tile
    from concourse import bass_isa, mybir

    f32 = mybir.dt.float32
    bf16 = mybir.dt.bfloat16
    AF = mybir.ActivationFunctionType
    Alu = mybir.AluOpType

    nc = bacc.Bacc("TRN2", target_bir_lowering=False, debug=False)

    x_ap = nc.dram_tensor("x", [B, T, C], f32, kind="ExternalInput").ap()
    y_ap = nc.dram_tensor("y", [B, OUT_COLS], f32, kind="ExternalOutput").ap()

    import contextlib

    with tile.TileContext(nc) as tc:
      for _rep in range(rep):
        loop_cm = tc.For_i(0, loop_n, 1) if loop_n else contextlib.nullcontext()
        with (
            loop_cm,
            tc.tile_pool(name="xin", bufs=xt_bufs) as xpool,
            tc.tile_pool(name="xsq", bufs=x2_bufs) as x2pool,
            tc.tile_pool(name="tree", bufs=1) as tree_pool,
            tc.tile_pool(name="rows", bufs=rows_bufs) as row_pool,
            tc.tile_pool(name="stat", bufs=stat_bufs) as stat_pool,
            tc.tile_pool(name="persist", bufs=1) as pers,
            tc.tile_pool(name="small", bufs=1) as small,
            tc.tile_pool(name="ps", bufs=2, space="PSUM") as pspool,
        ):
            # all-samples bf16 buffer (64 KiB/partition)
            XH = pers.tile([P, B, J, C], bf16, tag="XH")

            ones_f = small.tile([P, 1], f32, tag="ones_f")
            nc.vector.memset(ones_f[:], 1.0)
            ones_h = small.tile([P, 1], bf16, tag="ones_h")
            nc.scalar.copy(ones_h[:], ones_f[:])
            # warm the sqrt table set so the tail std-sqrt pays no table load
            sqrt_warm = small.tile([1, 1], f32, tag="sqrt_warm")
            nc.scalar.activation(sqrt_warm[:], ones_f[0:1, :], AF.Sqrt)

            OUT = small.tile([B, OUT_COLS], f32, tag="OUT")
            E = small.tile([B, 2, C], f32, tag="endpoints")
            S32 = small.tile([B, C], f32, tag="S32")
            Q32 = small.tile([B, C], f32, tag="Q32")
            # staged [max | -min] per sample in bf16, converted at the tail
            MM = small.tile([B, 2, C], bf16, tag="MM")
            TMP1 = small.tile([B, C], f32, tag="TMP1")
            TMP2 = small.tile([B, C], f32, tag="TMP2")

            partial = not (do_cast and do_reduce and do_mm and do_par
                           and do_endpoint) or n_grps < N_GRP
            if partial:
                nc.vector.memset(OUT[:], 0.0)
                nc.vector.memset(S32[:], 0.0)
                nc.vector.memset(Q32[:], 0.0)
                nc.vector.memset(MM[:].rearrange("b e c -> b (e c)"), 0.0)
                nc.vector.memset(E[:].rearrange("b e c -> b (e c)"), 0.0)

            # endpoint rows for slope: x[:, 0, :] and x[:, T-1, :]
            if do_endpoint:
                eq = nc.gpsimd if e_q7dma else nc.scalar
                eq.dma_start(out=E[:], in_=x_ap[:, 0 : T : T - 1, :])
            if slope_top:
                # slope immediately after E lands: next iteration's E load
                # then WARs against this early read, not the iteration tail
                nc.vector.tensor_sub(TMP1[:], E[:, 1, :], E[:, 0, :])
                nc.vector.tensor_scalar_mul(
                    OUT[:, 4 * C : 5 * C], TMP1[:], 1.0 / (T - 1)
                )

            ps_live = {}

            def trees(lo, hi):
                """Interleaved max/min trees over XH[:, lo:hi] + gpsimd fold."""
                n = hi - lo
                MxNeg = stat_pool.tile([P, 2, n, C], bf16, tag=f"MxNeg{n}",
                                       name=f"MxNeg{n}")
                Mn = tree_pool.tile([P, n, C], bf16, tag=f"Mn{n}",
                                    name=f"Mn{n}")
                lv = {}
                for d, (op, key) in enumerate(
                    ((Alu.max, "x"), (Alu.min, "n"))
                ):
                    tA = tree_pool.tile(
                        [P, n, J // 2, C], bf16, tag=f"tA{d}{n}",
                        name=f"tA{d}{n}",
                    )
                    tB = tree_pool.tile(
                        [P, n, J // 4, C], bf16, tag=f"tB{d}{n}",
                        name=f"tB{d}{n}",
                    )
                    tD = tree_pool.tile(
                        [P, n, J // 8, C], bf16, tag=f"tC{d}{n}",
                        name=f"tC{d}{n}",
                    )
                    lv[key] = (op, tA, tB, tD)
                # interleave the two dependent level-chains so the DVE
                # pipeline never stalls on a same-chain dependency
                for key in ("x", "n"):
                    op, tA, tB, tD = lv[key]
                    nc.vector.tensor_tensor(
                        out=tA[:], in0=XH[:, lo:hi, 0 : J // 2, :],
                        in1=XH[:, lo:hi, J // 2 :, :], op=op,
                    )
                for key in ("x", "n"):
                    op, tA, tB, tD = lv[key]
                    nc.vector.tensor_tensor(
                        out=tB[:], in0=tA[:, :, 0 : J // 4, :],
                        in1=tA[:, :, J // 4 :, :], op=op,
                    )
                for key in ("x", "n"):
                    op, tA, tB, tD = lv[key]
                    nc.vector.tensor_tensor(
                        out=tD[:], in0=tB[:, :, 0 : J // 8, :],
                        in1=tB[:, :, J // 8 :, :], op=op,
                    )
                for key in ("x", "n"):
                    op, tA, tB, tD = lv[key]
                    dst = MxNeg[:, 0] if key == "x" else Mn[:]
                    nc.vector.tensor_tensor(
                        out=dst[:, 0:n], in0=tD[:, :, 0, :],
                        in1=tD[:, :, 1, :], op=op,
                    )
                nc.vector.tensor_scalar_mul(MxNeg[:, 1], Mn[:], -1.0)
                if do_par:
                    ARxn = stat_pool.tile([P, 2 * n * C], bf16, tag=f"AR{n}",
                                          name=f"AR{n}")
                    nc.gpsimd.partition_all_reduce(
                        out_ap=ARxn[:],
                        in_ap=MxNeg[:].rearrange("p e s c -> p (e s c)"),
                        channels=P,
                        reduce_op=bass_isa.ReduceOp.max,
                    )
                    # ARxn[0] = [max(s0..), ..., negmin(s0..), ...]
                    mmq = nc.gpsimd if mm_q7dma else nc.scalar
                    mmq.dma_start(
                        out=MM[lo : lo + n, 0, :], in_=ARxn[0:1, 0 : n * C]
                    )
                    mmq.dma_start(
                        out=MM[lo : lo + n, 1, :],
                        in_=ARxn[0:1, n * C : 2 * n * C],
                    )

            for g in range(n_grps):
                xt = xpool.tile([P, S_GRP, J, C], f32, tag="xt")
                ldq = nc.scalar if (dma_split and g % 2) else nc.sync
                if batched_load:
                    src = x_ap[g * S_GRP : (g + 1) * S_GRP].rearrange(
                        "s (p j) c -> p s j c", p=P, j=J
                    )
                    ldq.dma_start(out=xt[:], in_=src)
                else:
                    for s in range(S_GRP):
                        src = x_ap[g * S_GRP + s].rearrange(
                            "(p j) c -> p j c", p=P, j=J
                        )
                        ldq.dma_start(out=xt[:, s], in_=src)

                lo = g * S_GRP
                xh_g = XH[:, lo : lo + S_GRP]
                if do_cast:
                    if g in dve_cast:
                        nc.vector.tensor_copy(out=xh_g, in_=xt[:])
                    else:
                        nc.scalar.copy(xh_g, xt[:])

                if do_cast and do_mm:
                    # squares from bf16 (xt is released after the cast)
                    x2 = x2pool.tile([P, S_GRP, J, C], bf16, tag="x2")
                    if g in dve_sq:
                        nc.vector.tensor_tensor(
                            out=x2[:], in0=xh_g, in1=xh_g, op=Alu.mult
                        )
                    else:
                        nc.scalar.activation(x2[:], xh_g, AF.Square)

                    h, half = divmod(g, 2)
                    if half == 0:
                        psS = pspool.tile([1, 2 * S_GRP * C], f32, tag="psS")
                        psQ = pspool.tile([1, 2 * S_GRP * C], f32, tag="psQ")
                        ps_live[h] = (psS, psQ)
                    else:
                        psS, psQ = ps_live.pop(h)
                    cols = bass.ts(half, S_GRP * C)
                    for j in range(J):
                        nc.tensor.matmul(
                            out=psS[:, cols],
                            lhsT=ones_h[:],
                            rhs=xh_g[:, :, j, :],
                            start=(j == 0),
                            stop=(j == J - 1),
                        )
                    for j in range(J):
                        nc.tensor.matmul(
                            out=psQ[:, cols],
                            lhsT=ones_h[:],
                            rhs=x2[:, :, j, :],
                            start=(j == 0),
                            stop=(j == J - 1),
                        )
                    if half == 1:
                        SR = row_pool.tile([1, 2 * S_GRP * C], f32, tag="SR")
                        QR = row_pool.tile([1, 2 * S_GRP * C], f32, tag="QR")
                        nc.scalar.copy(SR[:], psS[:])
                        nc.scalar.copy(QR[:], psQ[:])
                        srq = nc.gpsimd if srq_q7dma else nc.scalar
                        srq.dma_start(
                            out=S32[h * S_TREE : (h + 1) * S_TREE, :],
                            in_=SR[0:1, :],
                        )
                        srq.dma_start(
                            out=Q32[h * S_TREE : (h + 1) * S_TREE, :],
                            in_=QR[0:1, :],
                        )

                if do_cast and do_reduce and g % 2 == 1:
                    h = g // 2
                    lo_t = h * S_TREE
                    if split_last_tree and g == N_GRP - 1:
                        trees(lo_t, lo_t + S_GRP)
                        trees(lo_t + S_GRP, lo_t + S_TREE)
                    else:
                        trees(lo_t, lo_t + S_TREE)

            # max = MM[:, 0]; min = -MM[:, 1]
            nc.scalar.copy(OUT[:, 2 * C : 3 * C], MM[:, 0])
            nc.vector.tensor_scalar_mul(OUT[:, 3 * C : 4 * C], MM[:, 1], -1.0)

            # mean = S / T
            nc.vector.tensor_scalar_mul(OUT[:, 0:C], S32[:], 1.0 / T)
            # var = (Q - S * mean) / (T - 1); std = sqrt(var)
            nc.vector.tensor_tensor(
                out=TMP1[:], in0=S32[:], in1=OUT[:, 0:C], op=Alu.mult
            )
            nc.vector.tensor_sub(TMP2[:], Q32[:], TMP1[:])
            nc.vector.tensor_scalar_mul(TMP2[:], TMP2[:], 1.0 / (T - 1))
            nc.scalar.activation(OUT[:, C : 2 * C], TMP2[:], AF.Sqrt)

            if not slope_top:
                # slope = (x[:, -1, :] - x[:, 0, :]) / (T - 1)
                nc.vector.tensor_sub(TMP1[:], E[:, 1, :], E[:, 0, :])
                nc.vector.tensor_scalar_mul(
                    OUT[:, 4 * C : 5 * C], TMP1[:], 1.0 / (T - 1)
                )

            # keep y off the SP ring: otherwise the next iteration's bulk
            # loads queue behind it and stall until this tail completes
            yq = nc.gpsimd if y_q7dma else nc.sync
            yq.dma_start(out=y_ap, in_=OUT[:])

    nc.compile()
    return nc


def _get_nc():
    if getattr(_cache, "nc", None) is None:
        _cache.nc = _build()
    return _cache.nc


def kernel(x: np.ndarray) -> np.ndarray:
    from concourse.bass_utils import run_bass_kernel_spmd

    x = np.ascontiguousarray(x, dtype=np.float32)
    assert x.shape == (B_TOTAL, T, C), x.shape

    nc = _get_nc()
    in_maps = [{"x": x[k * B : (k + 1) * B]} for k in range(N_CORES)]
    last_err = None
    for _attempt in range(3):
        try:
            res = run_bass_kernel_spmd(nc, in_maps, list(range(N_CORES)))
            break
        except Exception as e:  # transient axon transfer errors — retry
            last_err = e
    else:
        raise last_err
    return np.concatenate([res.results[k]["y"] for k in range(N_CORES)], axis=0)
